# revision 6
# baseline (speedup 1.0000x reference)
"""AFT-Full (Attention Free Transformer) Trainium2 kernel.

Problem: nn_AFT_Full (B=8, H=W=128, C=512, fp32 io).

    q = x @ Wq + bq ; k = x @ Wk + bk ; v = x @ Wv + bv        (per-token C x C)
    ew = exp(w[:H, :W])                                         [H, W]
    num = einsum('iw,bhwc->bhic', ew, exp(k) * v)
    den = einsum('iw,bhwc->bhic', ew, exp(k))
    y   = sigmoid(q) * num / den
    out = y @ Wo + bo

Distribution: pure data-parallel over B -- one batch element per NeuronCore,
8 cores, no collectives.  Weights + the [128,128] position-bias slice are
replicated.

Per-core dataflow (per h; matmuls bf16 with fp32 PSUM accumulation, the
v-projection in fp8-e4m3 DoubleRow at 2 MACs/cell/cycle):

  - x^T arrives pre-transposed from the host as [c, w] chunks (bf16 + an fp8
    copy for the v matmuls), so C-contraction needs no on-device transpose.
  - q is computed TRANSPOSED (q^T [c, tok], Wq chunks stationary, 512-token
    blocks = 4 h-rows per PSUM bank, one c-chunk bank at a time), because the
    whole back end runs in the transposed layout:
  - k, v natural [w, c]; ek = exp(k) (ScalarE), ekv = ek*v (VectorE).
  - num^T/den^T [c, i] directly: lhsT = ekv/ek c-chunks (stationary),
    rhs = exp(w^T) streaming -- this replaces the natural num/den matmuls
    AND the 4 identity-transpose matmuls + PSUM->SBUF copy of y of the
    previous design at the same PE cost as num/den alone.
  - sigmoid(q) = 0.5*(1 + tanh(q/2)); tanh shares the ACT table set with exp
    so there is exactly one table load.  The 0.5 is folded into the num
    stationary (ewt_h = exp(w^T - ln2)).
  - y^T = (tanh(q^T/2) + 1) * num^T * reciprocal_approx_fast(den^T)  (VectorE)
  - out = (y^T).T @ Wo via 4 accumulating matmuls, natural [w, c] -> DMA out.

The per-h work is emitted as a software pipeline: per iteration t the PE sees
  nd(t-1) | qT-quarter(t+3) | k,v(t) | out(t-2)
so every matmul's producers are ~1 iteration old.
"""

import sys

if "/opt/trn_rl_repo" not in sys.path:
    sys.path.insert(0, "/opt/trn_rl_repo")

import ml_dtypes
import numpy as np

import concourse.bass as bass  # noqa: F401  (registers AP machinery)
import concourse.mybir as mybir
import concourse.tile as tile
from concourse import bacc
from concourse.bass_utils import run_bass_kernel_spmd

BF16 = mybir.dt.bfloat16
F32 = mybir.dt.float32
F8 = mybir.dt.float8e4
AF = mybir.ActivationFunctionType
OP = mybir.AluOpType
DR = mybir.MatmulPerfMode.DoubleRow

B, H, W, C = 8, 128, 128, 512
G = 8            # h-rows per input DMA group
NG = H // G      # 16 groups
NM = C // 128    # 4 contraction chunks

# 0: v-projection in bf16; 1: half the contraction in fp8 DoubleRow;
# 2: all of it in fp8 DoubleRow.  Verified absmax-rel error (vs f64 ref,
# same seeded inputs the harness uses): 2 -> 1.62e-2, 1 -> 1.36e-2,
# 0 -> 3.6e-3, against a 2e-2 gate.
V_MODE = 2

LAST_EXEC_NS = None
_NC_CACHE = {}


def _build_nc(has_bias: bool, v_mode: int):
    n8 = 2 * v_mode  # fp8 contraction chunks resident (0, 2 or 4)
    nc = bacc.Bacc(None, target_bir_lowering=False)

    xt_d = nc.dram_tensor("xt", [NG, NM, 128, G * 128], BF16, kind="ExternalInput")
    w_d = nc.dram_tensor("wqkvo", [128, 4, NM, 512], BF16, kind="ExternalInput")
    wt_d = nc.dram_tensor("wt", [128, 128], F32, kind="ExternalInput")
    if v_mode:
        xt8_d = nc.dram_tensor("xt8", [NG, n8, 128, G * 128], F8,
                               kind="ExternalInput")
        wv8_d = nc.dram_tensor("wv8", [128, n8, 512], F8, kind="ExternalInput")
    if has_bias:
        bqt_d = nc.dram_tensor("bqt", [128, NM], F32, kind="ExternalInput")
        bv_d = nc.dram_tensor("bvf", [128, C], F32, kind="ExternalInput")
        bo_d = nc.dram_tensor("bof", [128, C], F32, kind="ExternalInput")
    out_d = nc.dram_tensor("out", [H, 128, C], F32, kind="ExternalOutput")

    with tile.TileContext(nc) as tc:
        with (
            tc.tile_pool(name="const", bufs=1) as cpool,
            tc.tile_pool(name="xt", bufs=2) as xpool,
            tc.tile_pool(name="work", bufs=4) as wpool,
            tc.tile_pool(name="tq", bufs=2) as tqpool,
            tc.tile_pool(name="ps_qt", bufs=2, space="PSUM") as ps_qt,
            tc.tile_pool(name="ps_kv", bufs=3, space="PSUM") as ps_kv,
            tc.tile_pool(name="ps_nd", bufs=2, space="PSUM") as ps_nd,
            tc.tile_pool(name="ps_out", bufs=1, space="PSUM") as ps_out,
        ):
            gx = {}       # g -> group input tile [128, NM, G*128] bf16
            gx8 = {}      # g -> group input tile [128, n8, G*128] fp8
            st = {}       # h -> per-h state tiles
            tq = {}       # block -> tanh(q^T/2) tile [128, NM, 512]

            def load_group(g, split=False):
                gx[g] = xpool.tile([128, NM, G * 128], BF16, tag="gx", name="gx")
                src = xt_d[g].rearrange("m p r -> p m r")
                if v_mode:
                    gx8[g] = xpool.tile([128, n8, G * 128], F8, tag="gx8",
                                        name="gx8")
                    src8 = xt8_d[g].rearrange("m p r -> p m r")
                if split:
                    # stage the first 4-h block's tokens first so the t=0
                    # prologue can start as soon as possible
                    half = G * 128 // 2
                    nc.sync.dma_start(gx[g][:, :, :half], src[:, :, :half])
                else:
                    nc.sync.dma_start(gx[g][:], src)
                    if v_mode:
                        nc.sync.dma_start(gx8[g][:], src8[:])

            # ---- constants / startup ----
            # PE warm-up on an on-chip memset tile (no DMA dependency): keeps
            # the HAM clock-gate busy while the first input DMAs stream, so
            # real matmuls start at 8/8 (saves the ~4us cold-ramp).
            warm = cpool.tile([128, 512], BF16)
            nc.gpsimd.memset(warm[:], 0.0)
            warm_ps = ps_out.tile([128, 512], F32, tag="op", name="warm")
            for _ in range(12):
                nc.tensor.matmul(warm_ps[:], warm[:, :128], warm[:],
                                 start=True, stop=True)

            # DMA issue order == startup critical path order: first 4-h block
            # of x (bf16), Wq (t=0 q^T burst), Wk, the fp8 v operands, wt,
            # rest of group 0, Wo (needed at t=2), then the leftovers.
            w_sb = cpool.tile([128, 4, NM, 512], BF16)
            load_group(0, split=True)
            # Wq in c_out chunks so the first q^T unit only waits for 1/4 of
            # the weight bytes
            for j in range(NM):
                nc.sync.dma_start(w_sb[:, 0, :, j * 128:(j + 1) * 128],
                                  w_d[:, 0, :, j * 128:(j + 1) * 128])
            nc.sync.dma_start(w_sb[:, 1], w_d[:, 1])
            if v_mode:
                wv8_sb = cpool.tile([128, n8, 512], F8)
                nc.sync.dma_start(wv8_sb[:], wv8_d[:])
                nc.sync.dma_start(gx8[0][:, :, :512],
                                  xt8_d[0].rearrange("m p r -> p m r")[:, :, :512])
            wtmp = cpool.tile([128, 128], F32)
            nc.sync.dma_start(wtmp[:], wt_d[:])
            nc.sync.dma_start(w_sb[:, 3], w_d[:, 3])
            half = G * 128 // 2
            src0 = xt_d[0].rearrange("m p r -> p m r")
            nc.sync.dma_start(gx[0][:, :, half:], src0[:, :, half:])
            if v_mode:
                nc.sync.dma_start(gx8[0][:, :, 512:],
                                  xt8_d[0].rearrange("m p r -> p m r")[:, :, 512:])
            if v_mode < 2:
                nc.sync.dma_start(w_sb[:, 2], w_d[:, 2])

            ewt = cpool.tile([128, 128], BF16)
            nc.scalar.activation(ewt[:], wtmp[:], AF.Exp)
            # exp(wt - ln2) = 0.5*exp(wt); carries the 0.5 of
            # sigmoid(q) = 0.5*(1 + tanh(q/2)).
            ewt_h = cpool.tile([128, 128], BF16)
            nln2 = cpool.tile([128, 1], F32)
            nc.gpsimd.memset(nln2[:], -0.6931471805599453)
            nc.scalar.activation(ewt_h[:], wtmp[:], AF.Exp, bias=nln2[:])
            if has_bias:
                bqt_sb = cpool.tile([128, NM], F32)
                nc.sync.dma_start(bqt_sb[:], bqt_d[:])
                bv_sb = cpool.tile([128, C], F32)
                nc.sync.dma_start(bv_sb[:], bv_d[:])
                bo_sb = cpool.tile([128, C], F32)
                nc.sync.dma_start(bo_sb[:], bo_d[:])

            def qt_unit(u):
                """One c-chunk of the transposed q projection for a 4-h
                block: q^T[j-chunk, 512 tokens] + its tanh."""
                b, j = divmod(u, NM)
                g, rem = divmod(4 * b, G)
                toks = slice(rem * 128, rem * 128 + 512)
                if j == 0:
                    tq[b] = tqpool.tile([128, NM, 512], F32, tag="tq", name="tq")
                qp = ps_qt.tile([128, 512], F32, tag="qt", name="qt")
                for m in range(NM):
                    nc.tensor.matmul(qp[:], w_sb[:, 0, m, j * 128:(j + 1) * 128],
                                     gx[g][:, m, toks],
                                     start=(m == 0), stop=(m == NM - 1))
                if has_bias:
                    nc.scalar.activation(tq[b][:, j, :], qp[:], AF.Tanh,
                                         scale=0.5, bias=bqt_sb[:, j:j + 1])
                else:
                    nc.scalar.activation(tq[b][:, j, :], qp[:], AF.Tanh,
                                         scale=0.5)

            def stage_kv(h):
                """k/v projections + exp(k) + ek*v."""
                g, hg = divmod(h, G)
                if hg == 1 and g + 1 < NG:
                    load_group(g + 1)
                s = st[h] = {}
                hgs = slice(hg * 128, (hg + 1) * 128)
                k_ps = ps_kv.tile([128, 512], F32, tag="kv", name="kv")
                v_ps = ps_kv.tile([128, 512], F32, tag="kv", name="kv")
                for m in range(NM):
                    nc.tensor.matmul(k_ps[:], gx[g][:, m, hgs], w_sb[:, 1, m, :],
                                     start=(m == 0), stop=(m == NM - 1))
                if v_mode:
                    for p in range(v_mode):
                        nc.tensor.matmul(
                            v_ps[:], gx8[g][:, 2 * p:2 * p + 2, hgs],
                            wv8_sb[:, 2 * p:2 * p + 2, :],
                            start=(p == 0),
                            stop=(p == v_mode - 1 and v_mode == 2),
                            perf_mode=DR)
                    for m in range(n8, NM):
                        nc.tensor.matmul(v_ps[:], gx[g][:, m, hgs],
                                         w_sb[:, 2, m, :],
                                         start=False, stop=(m == NM - 1))
                else:
                    for m in range(NM):
                        nc.tensor.matmul(v_ps[:], gx[g][:, m, hgs],
                                         w_sb[:, 2, m, :],
                                         start=(m == 0), stop=(m == NM - 1))
                ek = s["ek"] = wpool.tile([128, 512], BF16, tag="ek", name="ek")
                nc.scalar.activation(ek[:], k_ps[:], AF.Exp)
                if has_bias:
                    vb = wpool.tile([128, 512], F32, tag="vb", name="vb")
                    nc.vector.tensor_add(out=vb[:], in0=v_ps[:], in1=bv_sb[:])
                    vsrc = vb
                else:
                    vsrc = v_ps
                ekv = s["ekv"] = wpool.tile([128, 512], BF16, tag="ekv",
                                            name="ekv")
                nc.vector.tensor_mul(out=ekv[:], in0=ek[:], in1=vsrc[:])

            def stage_nd(h):
                """num^T/den^T [c, i] via ekv/ek c-chunk stationaries; den
                first so the VectorE ratio can start while num streams."""
                s = st[h]
                dd = s["dd"] = ps_nd.tile([128, NM, 128], F32, tag="nd",
                                          name="nd")
                nd = s["nd"] = ps_nd.tile([128, NM, 128], F32, tag="nd",
                                          name="nd")
                for j in range(NM):
                    nc.tensor.matmul(dd[:, j, :],
                                     s["ek"][:, j * 128:(j + 1) * 128], ewt[:],
                                     start=(j == 0), stop=(j == NM - 1))
                for j in range(NM):
                    nc.tensor.matmul(nd[:, j, :],
                                     s["ekv"][:, j * 128:(j + 1) * 128],
                                     ewt_h[:],
                                     start=(j == 0), stop=(j == NM - 1))

            def stage_y(h):
                """Gated ratio on VectorE, in the transposed layout."""
                s = st[h]
                b, hg4 = divmod(h, 4)
                nd = s.pop("nd")
                dd = s.pop("dd")
                r = wpool.tile([128, NM, 128], F32, tag="r", name="r")
                nc.vector.reciprocal_approx_fast(out=r[:], in_=dd[:])
                y1 = wpool.tile([128, NM, 128], F32, tag="y1", name="y1")
                nc.vector.tensor_mul(out=y1[:], in0=nd[:], in1=r[:])
                yt = s["yt"] = wpool.tile([128, NM, 128], BF16, tag="yt",
                                          name="yt")
                # y^T = (tanh(q^T/2) + 1) * 0.5*num^T/den^T (0.5 in ewt_h)
                nc.vector.scalar_tensor_tensor(
                    out=yt[:], in0=tq[b][:, :, hg4 * 128:(hg4 + 1) * 128],
                    scalar=1.0, in1=y1[:], op0=OP.add, op1=OP.mult)
                del s["ek"], s["ekv"]

            def stage_out(h):
                """Output projection + store."""
                s = st.pop(h)
                o_ps = ps_out.tile([128, 512], F32, tag="op", name="op")
                for j in range(NM):
                    nc.tensor.matmul(o_ps[:], s["yt"][:, j, :],
                                     w_sb[:, 3, j, :],
                                     start=(j == 0), stop=(j == NM - 1))
                o_sb = wpool.tile([128, 512], F32, tag="o_sb", name="o_sb")
                if has_bias:
                    nc.vector.tensor_add(out=o_sb[:], in0=o_ps[:],
                                         in1=bo_sb[:])
                    nc.sync.dma_start(out_d[h], o_sb[:])
                elif h >= H - 2:
                    # split the drain-phase stores so the DMA overlaps the
                    # PSUM->SBUF copy
                    nc.scalar.copy(o_sb[:, :256], o_ps[:, :256])
                    nc.sync.dma_start(out_d[h, :, :256], o_sb[:, :256])
                    nc.scalar.copy(o_sb[:, 256:], o_ps[:, 256:])
                    nc.sync.dma_start(out_d[h, :, 256:], o_sb[:, 256:])
                else:
                    nc.scalar.copy(o_sb[:], o_ps[:])
                    nc.sync.dma_start(out_d[h], o_sb[:])

            for t in range(H + 2):
                if 1 <= t < H + 1:
                    stage_nd(t - 1)
                    stage_y(t - 1)
                if t == 0:
                    for u in range(NM):
                        qt_unit(u)
                elif t + 3 < H:
                    qt_unit(t + 3)
                if t < H:
                    stage_kv(t)
                if 2 <= t:
                    stage_out(t - 2)

    nc.compile()
    return nc


def _prep_core_arr(xb):
    """x[b] [H, W, C] f32 -> [NG, NM, 128, G*128] pre-transposed layout."""
    a = xb.transpose(2, 0, 1)                    # [c, h, w]
    a = a.reshape(NM, 128, NG, G, W)             # [m, c_sub, g, hg, w]
    a = a.transpose(2, 0, 1, 3, 4)               # [g, m, c_sub, hg, w]
    return a.reshape(NG, NM, 128, G * W)


def kernel(x, Wq, bq, Wk, bk, Wv, bv, w, Wo, bo, _profile=False):
    global LAST_EXEC_NS
    x = np.asarray(x, dtype=np.float32)
    assert x.shape == (B, H, W, C), x.shape

    # bk cancels exactly in num/den; bq, bv, bo need extra work only if
    # nonzero.
    has_bias = bool(np.any(np.asarray(bq)) or np.any(np.asarray(bv))
                    or np.any(np.asarray(bo)))
    v_mode = V_MODE
    n8 = 2 * v_mode

    key = (has_bias, v_mode)
    if key not in _NC_CACHE:
        _NC_CACHE[key] = _build_nc(has_bias, v_mode)
    nc = _NC_CACHE[key]

    wq4 = np.stack([np.asarray(Wq), np.asarray(Wk), np.asarray(Wv),
                    np.asarray(Wo)]).astype(np.float32)   # [4, C, C]
    wq4 = wq4.reshape(4, NM, 128, C).transpose(2, 0, 1, 3)  # [c_sub, which, m, c]
    w_host = np.ascontiguousarray(wq4).astype(ml_dtypes.bfloat16)
    wt_host = np.ascontiguousarray(np.asarray(w)[:H, :W].T).astype(np.float32)

    base = {"wqkvo": w_host, "wt": wt_host}
    if v_mode:
        wv8 = np.asarray(Wv, np.float32).reshape(NM, 128, C).transpose(1, 0, 2)
        wv8 = np.clip(wv8[:, :n8], -240, 240)
        base["wv8"] = np.ascontiguousarray(wv8).astype(ml_dtypes.float8_e4m3)
    if has_bias:
        bqt = 0.5 * np.asarray(bq, np.float32).reshape(NM, 128).T
        base["bqt"] = np.ascontiguousarray(bqt)
        base["bvf"] = np.ascontiguousarray(
            np.broadcast_to(np.asarray(bv, np.float32), (128, C)))
        base["bof"] = np.ascontiguousarray(
            np.broadcast_to(np.asarray(bo, np.float32), (128, C)))

    in_maps = []
    for b in range(B):
        a = _prep_core_arr(x[b])
        m = dict(base, xt=np.ascontiguousarray(a).astype(ml_dtypes.bfloat16))
        if v_mode:
            m["xt8"] = np.ascontiguousarray(
                np.clip(a[:, :n8], -240, 240)).astype(ml_dtypes.float8_e4m3)
        in_maps.append(m)

    res = run_bass_kernel_spmd(nc, in_maps, core_ids=list(range(B)),
                               trace=bool(_profile))
    LAST_EXEC_NS = res.exec_time_ns
    globals()["LAST_RESULT"] = res
    return np.stack([res.results[b]["out"] for b in range(B)]).astype(np.float32)


# revision 8
# speedup vs baseline: 1.0039x; 1.0039x over previous
"""AFT-Full (Attention Free Transformer) Trainium2 kernel.

Problem: nn_AFT_Full (B=8, H=W=128, C=512, fp32 io).

    q = x @ Wq + bq ; k = x @ Wk + bk ; v = x @ Wv + bv        (per-token C x C)
    ew = exp(w[:H, :W])                                         [H, W]
    num = einsum('iw,bhwc->bhic', ew, exp(k) * v)
    den = einsum('iw,bhwc->bhic', ew, exp(k))
    y   = sigmoid(q) * num / den
    out = y @ Wo + bo

Distribution: pure data-parallel over B -- one batch element per NeuronCore,
8 cores, no collectives.  Weights + the [128,128] position-bias slice are
replicated.

Per-core dataflow (per h; matmuls bf16 with fp32 PSUM accumulation, the
v-projection in fp8-e4m3 DoubleRow at 2 MACs/cell/cycle):

  - x^T arrives pre-transposed from the host as [c, w] chunks (bf16 + an fp8
    copy for the v matmuls), so C-contraction needs no on-device transpose.
  - q is computed TRANSPOSED (q^T [c, tok], Wq chunks stationary, 512-token
    blocks = 4 h-rows per PSUM bank, one c-chunk bank at a time), because the
    whole back end runs in the transposed layout:
  - k, v natural [w, c]; ek = exp(k) (ScalarE), ekv = ek*v (VectorE).
  - num^T/den^T [c, i] directly: lhsT = ekv/ek c-chunks (stationary),
    rhs = exp(w^T) streaming -- this replaces the natural num/den matmuls
    AND the 4 identity-transpose matmuls + PSUM->SBUF copy of y of the
    previous design at the same PE cost as num/den alone.
  - sigmoid(q) = 0.5*(1 + tanh(q/2)); tanh shares the ACT table set with exp
    so there is exactly one table load.  The 0.5 is folded into the num
    stationary (ewt_h = exp(w^T - ln2)).
  - y^T = (tanh(q^T/2) + 1) * num^T * reciprocal_approx_fast(den^T)  (VectorE)
  - out = (y^T).T @ Wo via 4 accumulating matmuls, natural [w, c] -> DMA out.

The per-h work is emitted as a software pipeline: per iteration t the PE sees
  nd(t-1) | qT-quarter(t+3) | k,v(t) | out(t-2)
so every matmul's producers are ~1 iteration old.
"""

import sys

if "/opt/trn_rl_repo" not in sys.path:
    sys.path.insert(0, "/opt/trn_rl_repo")

import ml_dtypes
import numpy as np

import concourse.bass as bass  # noqa: F401  (registers AP machinery)
import concourse.mybir as mybir
import concourse.tile as tile
from concourse import bacc
from concourse.bass_utils import run_bass_kernel_spmd

BF16 = mybir.dt.bfloat16
F32 = mybir.dt.float32
F8 = mybir.dt.float8e4
AF = mybir.ActivationFunctionType
OP = mybir.AluOpType
DR = mybir.MatmulPerfMode.DoubleRow

B, H, W, C = 8, 128, 128, 512
G = 8            # h-rows per input DMA group
NG = H // G      # 16 groups
NM = C // 128    # 4 contraction chunks

# 0: v-projection in bf16; 1: half the contraction in fp8 DoubleRow;
# 2: all of it in fp8 DoubleRow.  Verified absmax-rel error (vs f64 ref,
# same seeded inputs the harness uses): 2 -> 1.62e-2, 1 -> 1.36e-2,
# 0 -> 3.6e-3, against a 2e-2 gate.
V_MODE = 2

LAST_EXEC_NS = None
_NC_CACHE = {}


def _build_nc(has_bias: bool, v_mode: int):
    n8 = 2 * v_mode  # fp8 contraction chunks resident (0, 2 or 4)
    nc = bacc.Bacc(None, target_bir_lowering=False)

    xt_d = nc.dram_tensor("xt", [NG, NM, 128, G * 128], BF16, kind="ExternalInput")
    w_d = nc.dram_tensor("wqkvo", [128, 4, NM, 512], BF16, kind="ExternalInput")
    wt_d = nc.dram_tensor("wt", [128, 128], F32, kind="ExternalInput")
    if v_mode:
        xt8_d = nc.dram_tensor("xt8", [NG, n8, 128, G * 128], F8,
                               kind="ExternalInput")
        wv8_d = nc.dram_tensor("wv8", [128, n8, 512], F8, kind="ExternalInput")
    if has_bias:
        bqt_d = nc.dram_tensor("bqt", [128, NM], F32, kind="ExternalInput")
        bv_d = nc.dram_tensor("bvf", [128, C], F32, kind="ExternalInput")
        bo_d = nc.dram_tensor("bof", [128, C], F32, kind="ExternalInput")
    out_d = nc.dram_tensor("out", [H, 128, C], F32, kind="ExternalOutput")

    with tile.TileContext(nc) as tc:
        with (
            tc.tile_pool(name="const", bufs=1) as cpool,
            tc.tile_pool(name="xt", bufs=2) as xpool,
            tc.tile_pool(name="work", bufs=4) as wpool,
            tc.tile_pool(name="tq", bufs=2) as tqpool,
            tc.tile_pool(name="ps_qt", bufs=2, space="PSUM") as ps_qt,
            tc.tile_pool(name="ps_kv", bufs=3, space="PSUM") as ps_kv,
            tc.tile_pool(name="ps_nd", bufs=2, space="PSUM") as ps_nd,
            tc.tile_pool(name="ps_out", bufs=1, space="PSUM") as ps_out,
        ):
            gx = {}       # g -> group input tile [128, NM, G*128] bf16
            gx8 = {}      # g -> group input tile [128, n8, G*128] fp8
            st = {}       # h -> per-h state tiles
            tq = {}       # block -> tanh(q^T/2) tile [128, NM, 512]

            def load_group(g, split=False):
                gx[g] = xpool.tile([128, NM, G * 128], BF16, tag="gx", name="gx")
                src = xt_d[g].rearrange("m p r -> p m r")
                if v_mode:
                    gx8[g] = xpool.tile([128, n8, G * 128], F8, tag="gx8",
                                        name="gx8")
                    src8 = xt8_d[g].rearrange("m p r -> p m r")
                if split:
                    # stage the first 4-h block's tokens first so the t=0
                    # prologue can start as soon as possible
                    half = G * 128 // 2
                    nc.sync.dma_start(gx[g][:, :, :half], src[:, :, :half])
                else:
                    nc.sync.dma_start(gx[g][:], src)
                    if v_mode:
                        nc.sync.dma_start(gx8[g][:], src8[:])

            # ---- constants / startup ----
            # PE warm-up on an on-chip memset tile (no DMA dependency): keeps
            # the HAM clock-gate busy while the first input DMAs stream, so
            # real matmuls start at 8/8 (saves the ~4us cold-ramp).
            warm = cpool.tile([128, 512], BF16)
            nc.gpsimd.memset(warm[:], 0.0)
            warm_ps = ps_out.tile([128, 512], F32, tag="op", name="warm")
            for _ in range(10):
                nc.tensor.matmul(warm_ps[:], warm[:, :128], warm[:],
                                 start=True, stop=True)

            # DMA issue order == startup critical path order: first 4-h block
            # of x (bf16), Wq (t=0 q^T burst), Wk, the fp8 v operands, wt,
            # rest of group 0, Wo (needed at t=2), then the leftovers.
            w_sb = cpool.tile([128, 4, NM, 512], BF16)
            load_group(0, split=True)
            # Wq in c_out chunks so the first q^T unit only waits for 1/4 of
            # the weight bytes
            for j in range(NM):
                nc.sync.dma_start(w_sb[:, 0, :, j * 128:(j + 1) * 128],
                                  w_d[:, 0, :, j * 128:(j + 1) * 128])
            nc.sync.dma_start(w_sb[:, 1], w_d[:, 1])
            if v_mode:
                wv8_sb = cpool.tile([128, n8, 512], F8)
                nc.sync.dma_start(wv8_sb[:], wv8_d[:])
                nc.sync.dma_start(gx8[0][:, :, :512],
                                  xt8_d[0].rearrange("m p r -> p m r")[:, :, :512])
            wtmp = cpool.tile([128, 128], F32)
            nc.sync.dma_start(wtmp[:], wt_d[:])
            nc.sync.dma_start(w_sb[:, 3], w_d[:, 3])
            half = G * 128 // 2
            src0 = xt_d[0].rearrange("m p r -> p m r")
            nc.sync.dma_start(gx[0][:, :, half:], src0[:, :, half:])
            if v_mode:
                nc.sync.dma_start(gx8[0][:, :, 512:],
                                  xt8_d[0].rearrange("m p r -> p m r")[:, :, 512:])
            if v_mode < 2:
                nc.sync.dma_start(w_sb[:, 2], w_d[:, 2])

            ewt = cpool.tile([128, 128], BF16)
            nc.scalar.activation(ewt[:], wtmp[:], AF.Exp)
            # exp(wt - ln2) = 0.5*exp(wt); carries the 0.5 of
            # sigmoid(q) = 0.5*(1 + tanh(q/2)).
            ewt_h = cpool.tile([128, 128], BF16)
            nln2 = cpool.tile([128, 1], F32)
            nc.gpsimd.memset(nln2[:], -0.6931471805599453)
            nc.scalar.activation(ewt_h[:], wtmp[:], AF.Exp, bias=nln2[:])
            if has_bias:
                bqt_sb = cpool.tile([128, NM], F32)
                nc.sync.dma_start(bqt_sb[:], bqt_d[:])
                bv_sb = cpool.tile([128, C], F32)
                nc.sync.dma_start(bv_sb[:], bv_d[:])
                bo_sb = cpool.tile([128, C], F32)
                nc.sync.dma_start(bo_sb[:], bo_d[:])

            def qt_unit(u):
                """One c-chunk of the transposed q projection for a 4-h
                block: q^T[j-chunk, 512 tokens] + its tanh."""
                b, j = divmod(u, NM)
                g, rem = divmod(4 * b, G)
                toks = slice(rem * 128, rem * 128 + 512)
                if j == 0:
                    tq[b] = tqpool.tile([128, NM, 512], F32, tag="tq", name="tq")
                qp = ps_qt.tile([128, 512], F32, tag="qt", name="qt")
                for m in range(NM):
                    nc.tensor.matmul(qp[:], w_sb[:, 0, m, j * 128:(j + 1) * 128],
                                     gx[g][:, m, toks],
                                     start=(m == 0), stop=(m == NM - 1))
                if has_bias:
                    nc.scalar.activation(tq[b][:, j, :], qp[:], AF.Tanh,
                                         scale=0.5, bias=bqt_sb[:, j:j + 1])
                else:
                    nc.scalar.activation(tq[b][:, j, :], qp[:], AF.Tanh,
                                         scale=0.5)

            def stage_kv(h):
                """k/v projections + exp(k) + ek*v."""
                g, hg = divmod(h, G)
                if hg == 1 and g + 1 < NG:
                    load_group(g + 1)
                s = st[h] = {}
                hgs = slice(hg * 128, (hg + 1) * 128)
                k_ps = ps_kv.tile([128, 512], F32, tag="kv", name="kv")
                v_ps = ps_kv.tile([128, 512], F32, tag="kv", name="kv")
                for m in range(NM):
                    nc.tensor.matmul(k_ps[:], gx[g][:, m, hgs], w_sb[:, 1, m, :],
                                     start=(m == 0), stop=(m == NM - 1))
                if v_mode:
                    for p in range(v_mode):
                        nc.tensor.matmul(
                            v_ps[:], gx8[g][:, 2 * p:2 * p + 2, hgs],
                            wv8_sb[:, 2 * p:2 * p + 2, :],
                            start=(p == 0),
                            stop=(p == v_mode - 1 and v_mode == 2),
                            perf_mode=DR)
                    for m in range(n8, NM):
                        nc.tensor.matmul(v_ps[:], gx[g][:, m, hgs],
                                         w_sb[:, 2, m, :],
                                         start=False, stop=(m == NM - 1))
                else:
                    for m in range(NM):
                        nc.tensor.matmul(v_ps[:], gx[g][:, m, hgs],
                                         w_sb[:, 2, m, :],
                                         start=(m == 0), stop=(m == NM - 1))
                ek = s["ek"] = wpool.tile([128, 512], BF16, tag="ek", name="ek")
                nc.scalar.activation(ek[:], k_ps[:], AF.Exp)
                if has_bias:
                    vb = wpool.tile([128, 512], F32, tag="vb", name="vb")
                    nc.vector.tensor_add(out=vb[:], in0=v_ps[:], in1=bv_sb[:])
                    vsrc = vb
                else:
                    vsrc = v_ps
                ekv = s["ekv"] = wpool.tile([128, 512], BF16, tag="ekv",
                                            name="ekv")
                nc.vector.tensor_mul(out=ekv[:], in0=ek[:], in1=vsrc[:])

            def stage_nd(h):
                """num^T/den^T [c, i] via ekv/ek c-chunk stationaries; den
                first so the VectorE ratio can start while num streams."""
                s = st[h]
                dd = s["dd"] = ps_nd.tile([128, NM, 128], F32, tag="nd",
                                          name="nd")
                nd = s["nd"] = ps_nd.tile([128, NM, 128], F32, tag="nd",
                                          name="nd")
                for j in range(NM):
                    nc.tensor.matmul(dd[:, j, :],
                                     s["ek"][:, j * 128:(j + 1) * 128], ewt[:],
                                     start=(j == 0), stop=(j == NM - 1))
                for j in range(NM):
                    nc.tensor.matmul(nd[:, j, :],
                                     s["ekv"][:, j * 128:(j + 1) * 128],
                                     ewt_h[:],
                                     start=(j == 0), stop=(j == NM - 1))

            def stage_y(h):
                """Gated ratio on VectorE, in the transposed layout."""
                s = st[h]
                b, hg4 = divmod(h, 4)
                nd = s.pop("nd")
                dd = s.pop("dd")
                r = wpool.tile([128, NM, 128], F32, tag="r", name="r")
                nc.vector.reciprocal_approx_fast(out=r[:], in_=dd[:])
                y1 = wpool.tile([128, NM, 128], F32, tag="y1", name="y1")
                nc.vector.tensor_mul(out=y1[:], in0=nd[:], in1=r[:])
                yt = s["yt"] = wpool.tile([128, NM, 128], BF16, tag="yt",
                                          name="yt")
                # y^T = (tanh(q^T/2) + 1) * 0.5*num^T/den^T (0.5 in ewt_h)
                nc.vector.scalar_tensor_tensor(
                    out=yt[:], in0=tq[b][:, :, hg4 * 128:(hg4 + 1) * 128],
                    scalar=1.0, in1=y1[:], op0=OP.add, op1=OP.mult)
                del s["ek"], s["ekv"]

            def stage_out(h):
                """Output projection + store."""
                s = st.pop(h)
                o_ps = ps_out.tile([128, 512], F32, tag="op", name="op")
                for j in range(NM):
                    nc.tensor.matmul(o_ps[:], s["yt"][:, j, :],
                                     w_sb[:, 3, j, :],
                                     start=(j == 0), stop=(j == NM - 1))
                o_sb = wpool.tile([128, 512], F32, tag="o_sb", name="o_sb")
                if has_bias:
                    nc.vector.tensor_add(out=o_sb[:], in0=o_ps[:],
                                         in1=bo_sb[:])
                    nc.sync.dma_start(out_d[h], o_sb[:])
                elif h >= H - 2:
                    # split the drain-phase stores so the DMA overlaps the
                    # PSUM->SBUF copy
                    nc.scalar.copy(o_sb[:, :256], o_ps[:, :256])
                    nc.sync.dma_start(out_d[h, :, :256], o_sb[:, :256])
                    nc.scalar.copy(o_sb[:, 256:], o_ps[:, 256:])
                    nc.sync.dma_start(out_d[h, :, 256:], o_sb[:, 256:])
                else:
                    nc.scalar.copy(o_sb[:], o_ps[:])
                    nc.sync.dma_start(out_d[h], o_sb[:])

            for t in range(H + 2):
                if 1 <= t < H + 1:
                    stage_nd(t - 1)
                    stage_y(t - 1)
                if t == 0:
                    for u in range(NM):
                        qt_unit(u)
                elif t + 3 < H:
                    qt_unit(t + 3)
                if t < H:
                    stage_kv(t)
                if 2 <= t:
                    stage_out(t - 2)

    nc.compile()
    return nc


def _prep_core_arr(xb):
    """x[b] [H, W, C] f32 -> [NG, NM, 128, G*128] pre-transposed layout."""
    a = xb.transpose(2, 0, 1)                    # [c, h, w]
    a = a.reshape(NM, 128, NG, G, W)             # [m, c_sub, g, hg, w]
    a = a.transpose(2, 0, 1, 3, 4)               # [g, m, c_sub, hg, w]
    return a.reshape(NG, NM, 128, G * W)


def kernel(x, Wq, bq, Wk, bk, Wv, bv, w, Wo, bo, _profile=False):
    global LAST_EXEC_NS
    x = np.asarray(x, dtype=np.float32)
    assert x.shape == (B, H, W, C), x.shape

    # bk cancels exactly in num/den; bq, bv, bo need extra work only if
    # nonzero.
    has_bias = bool(np.any(np.asarray(bq)) or np.any(np.asarray(bv))
                    or np.any(np.asarray(bo)))
    v_mode = V_MODE
    n8 = 2 * v_mode

    key = (has_bias, v_mode)
    if key not in _NC_CACHE:
        _NC_CACHE[key] = _build_nc(has_bias, v_mode)
    nc = _NC_CACHE[key]

    wq4 = np.stack([np.asarray(Wq), np.asarray(Wk), np.asarray(Wv),
                    np.asarray(Wo)]).astype(np.float32)   # [4, C, C]
    wq4 = wq4.reshape(4, NM, 128, C).transpose(2, 0, 1, 3)  # [c_sub, which, m, c]
    w_host = np.ascontiguousarray(wq4).astype(ml_dtypes.bfloat16)
    wt_host = np.ascontiguousarray(np.asarray(w)[:H, :W].T).astype(np.float32)

    base = {"wqkvo": w_host, "wt": wt_host}
    if v_mode:
        wv8 = np.asarray(Wv, np.float32).reshape(NM, 128, C).transpose(1, 0, 2)
        wv8 = np.clip(wv8[:, :n8], -240, 240)
        base["wv8"] = np.ascontiguousarray(wv8).astype(ml_dtypes.float8_e4m3)
    if has_bias:
        bqt = 0.5 * np.asarray(bq, np.float32).reshape(NM, 128).T
        base["bqt"] = np.ascontiguousarray(bqt)
        base["bvf"] = np.ascontiguousarray(
            np.broadcast_to(np.asarray(bv, np.float32), (128, C)))
        base["bof"] = np.ascontiguousarray(
            np.broadcast_to(np.asarray(bo, np.float32), (128, C)))

    in_maps = []
    for b in range(B):
        a = _prep_core_arr(x[b])
        m = dict(base, xt=np.ascontiguousarray(a).astype(ml_dtypes.bfloat16))
        if v_mode:
            m["xt8"] = np.ascontiguousarray(
                np.clip(a[:, :n8], -240, 240)).astype(ml_dtypes.float8_e4m3)
        in_maps.append(m)

    try:
        res = run_bass_kernel_spmd(nc, in_maps, core_ids=list(range(B)),
                                   trace=bool(_profile))
    except Exception:
        # transient device errors (NRT_EXEC_UNIT_UNRECOVERABLE etc.) are
        # occasionally seen on back-to-back runs; one retry usually clears
        import time

        time.sleep(2.0)
        res = run_bass_kernel_spmd(nc, in_maps, core_ids=list(range(B)),
                                   trace=bool(_profile))
    LAST_EXEC_NS = res.exec_time_ns
    globals()["LAST_RESULT"] = res
    return np.stack([res.results[b]["out"] for b in range(B)]).astype(np.float32)
